# revision 53
# baseline (speedup 1.0000x reference)
"""GPT-2 small (L=12, C=768, H=12, T=1024, B=4) forward on 8 trn2 NeuronCores.

Sharding: data-parallel over batch (4 elems) x 2-way vocab shard of lm_head.
Core c handles batch elem c%4, vocab half c//4.

Weight delivery: every core needs the full layer weights (DP), but shipping 8
replicas host->device dominates the wall clock. Instead each core receives a
1/8 row-shard of the packed per-layer weight bundle and the full set is
reconstructed on device with one AllGather per layer PAIR (rank r holds rows
16r:16r+16 of each layer block; the rank-major gathered [8, 32, XL] tensor
exposes each layer's packed [128, XL] view as a 3D slice). Pairing rides the
collective bandwidth size-ramp and halves the per-collective floors. The lm
head is gathered
within 4-rank groups (one group per vocab half). A collective trigger holds
the issuing gpsimd queue for the collective's duration, so the gathers are
emitted interleaved with the layer loop (depth-1 prefetch) and all gpsimd
broadcasts were replaced with K=1 PE matmuls; the first lm weight loads carry
an explicit ordering edge so the scheduler cannot hoist them into a DMA queue
ahead of the layer loads (their AllGather lands last - FIFO head-of-line).

On-device layout: residual stream kept TRANSPOSED h_T [C(part), T(free)] as
6 tiles [128, 1024] f32. All matmuls contract over the partition dim; weights
stream from HBM in small stationary blocks. LayerNorm stats via ones-vector
matmuls on a bf16 mirror; (x-mu)*rstd applied through row broadcasts done as
ones-column (K=1) matmuls into PSUM. Attention computes transposed scores
directly (K stationary); the softmax denominator comes from an extra ones
column appended per-head to V. Logits are quantized on device to uint8 with
a per-(token, 512-vocab-chunk) scale (amax/127; the hw float->int convert
rounds to nearest) and dequantized on host, halving the output traffic
versus bf16 at ~7e-3 added relative error.
"""

import sys
import time
import numpy as np

for _p in ("/opt/trn_rl_repo", "/root/.axon_site/_ro/trn_rl_repo"):
    if _p not in sys.path:
        sys.path.insert(0, _p)

import ml_dtypes

BF16 = ml_dtypes.bfloat16

B, T, L, H, C = 4, 1024, 12, 12, 768
D = C // H
F = 4 * C
V = 50257
VPAD = 51200
VSH = VPAD // 2
CB = C // 128          # 6
FBL = F // 128         # 24
TT = T // 128          # 8
NTG = T // 512         # 2
NVC = VSH // 512       # 50
EPS = 1e-5

XQK = 12 * CB * 128    # 9216  per-layer packed col width
XV = CB * 768          # 4608
XPJ = CB * CB * 128    # 4608
XFC = FBL * CB * 128   # 18432
XMP = CB * FBL * 128   # 18432
XLM = NVC * CB * 512   # 153600
# one bundled [128, XL] block per layer: qk | v | proj | fc | mproj
OQK, OV, OPJ, OFC, OMP = (0, XQK, XQK + XV, XQK + XV + XPJ,
                          XQK + XV + XPJ + XFC)
XL = XQK + XV + XPJ + XFC + XMP  # 55296

_CACHE = {}


def _build(reps=1, use_ag=True):
    import concourse.bass as bass
    import concourse.mybir as mybir
    import concourse.tile as tile
    from concourse.tile import add_dep_helper
    from concourse import bacc
    from contextlib import ExitStack

    f32 = mybir.dt.float32
    bf16 = mybir.dt.bfloat16
    u8 = mybir.dt.uint8
    AF = mybir.ActivationFunctionType
    ALU = mybir.AluOpType
    AX = mybir.AxisListType
    ds = bass.ds

    nc = bacc.Bacc("TRN2", target_bir_lowering=False, debug=False,
                   enable_asserts=False, num_devices=8)

    h0 = nc.dram_tensor("h0", [128, CB * T], bf16, kind="ExternalInput").ap()
    if use_ag:
        # row-blocked so each layer's shard is a contiguous slice (collective
        # inputs must be contiguous): rows [16l:16l+16] = this core's 16 rows
        # of layer l's packed [128, XL] block.
        wl_s = nc.dram_tensor("wl", [L * 16, XL], bf16, kind="ExternalInput").ap()
        wlm_s = nc.dram_tensor("wlm", [32, XLM], bf16, kind="ExternalInput").ap()
    else:
        wl_f = nc.dram_tensor("wl", [128, L * XL], bf16, kind="ExternalInput").ap()
        wlm_f = nc.dram_tensor("wlm", [128, XLM], bf16, kind="ExternalInput").ap()
    lnp = nc.dram_tensor("lnp", [128, (4 * L + 2) * CB], f32, kind="ExternalInput").ap()
    # logits quantized to uint8 (q = round(x/scale) + 128) + per-(token,
    # 512-col-chunk) dequant scales; host reconstructs x = (q-128)*scale.
    out = nc.dram_tensor("out", [T, VSH], u8, kind="ExternalOutput").ap()
    osc = nc.dram_tensor("osc", [T, NVC], f32, kind="ExternalOutput").ap()

    with tile.TileContext(nc) as tc, ExitStack() as ctx:
        const = ctx.enter_context(tc.tile_pool(name="const", bufs=1))
        ph = ctx.enter_context(tc.tile_pool(name="ph", bufs=1))
        phb = ctx.enter_context(tc.tile_pool(name="phb", bufs=1))
        phn = ctx.enter_context(tc.tile_pool(name="phn", bufs=1))
        pqk = ctx.enter_context(tc.tile_pool(name="pqk", bufs=1))
        pv = ctx.enter_context(tc.tile_pool(name="pv", bufs=1))
        py = ctx.enter_context(tc.tile_pool(name="py", bufs=1))
        pg = ctx.enter_context(tc.tile_pool(name="pg", bufs=1))
        pexp = ctx.enter_context(tc.tile_pool(name="pexp", bufs=2))
        psq = ctx.enter_context(tc.tile_pool(name="psq", bufs=2))
        prow = ctx.enter_context(tc.tile_pool(name="prow", bufs=1))
        piz = ctx.enter_context(tc.tile_pool(name="piz", bufs=1))
        pbc = ctx.enter_context(tc.tile_pool(name="pbc", bufs=2))
        pwst = ctx.enter_context(tc.tile_pool(name="pwst", bufs=4))
        pwm = ctx.enter_context(tc.tile_pool(name="pwm", bufs=2))
        pwv = ctx.enter_context(tc.tile_pool(name="pwv", bufs=1))
        plm = ctx.enter_context(tc.tile_pool(name="plm", bufs=2))
        pmm = ctx.enter_context(tc.tile_pool(name="pmm", bufs=6, space="PSUM"))
        pst = ctx.enter_context(tc.tile_pool(name="pst", bufs=1, space="PSUM"))
        pd = ctx.enter_context(tc.tile_pool(name="pd", bufs=1, space="DRAM"))

        # ---- weight reconstruction via AllGather ----
        # Two layers per gather: bigger transfers ride the collective
        # bandwidth ramp and halve the per-collective floors. The gathered
        # tensor is rank-major [8, 32, XL]; layer 2p+k's [128, XL] view is
        # the 3D slice [:, 16k:16k+16, :] (8 x 16 partition rows = 128).
        # The collective trigger holds the issuing gpsimd queue for the
        # collective's duration, so gpsimd carries nothing else (broadcasts
        # run as K=1 PE matmuls) and gathers are emitted interleaved with
        # the layer loop to keep delivery just ahead of demand.
        gl_ = [None] * (L // 2)
        glm = None if use_ag else wlm_f

        def emit_ag(p):
            # collectives cannot read IO tensors: bounce through internal DRAM
            bnc = pd.tile([32, XL], bf16, tag=f"bw{p}", name=f"bw{p}")
            nc.sync.dma_start(bnc[:], wl_s[ds(p * 32, 32), :])
            g = pd.tile([8, 32, XL], bf16, tag=f"gw{p}",
                        name=f"gw{p}", addr_space="Shared")
            nc.gpsimd.collective_compute(
                "AllGather", ALU.bypass,
                replica_groups=[list(range(8))],
                ins=[bnc[:]], outs=[g[:]])
            gl_[p] = g

        def emit_ag_lm():
            blm = pd.tile([32, XLM], bf16, name="blm")
            nc.sync.dma_start(blm[:], wlm_s[:])
            glm_t = pd.tile([128, XLM], bf16, name="glm")
            nc.gpsimd.collective_compute(
                "AllGather", ALU.bypass,
                replica_groups=[[0, 1, 2, 3], [4, 5, 6, 7]],
                ins=[blm[:]], outs=[glm_t[:]])
            return glm_t

        def wsl(l, off, n):
            if use_ag:
                return gl_[l // 2][:, ds((l % 2) * 16, 16), ds(off, n)]
            return wl_f[:, ds(l * XL + off, n)]

        if use_ag:
            emit_ag(0)

        ones = const.tile([128, 1], bf16, tag="ones", name="ones")
        nc.vector.memset(ones[:], 1.0)
        # [1,128] ones row: K=1 stationary for PE row-broadcast matmuls
        onesr = const.tile([1, 128], f32, tag="onesr", name="onesr")
        nc.vector.memset(onesr[:], 1.0)
        eps1 = const.tile([1, 1], f32, tag="eps1", name="eps1")
        nc.vector.memset(eps1[:], EPS)
        # causal masks generated on device: masks[p, 512k+f] = (f - p >= 128k)
        masks = const.tile([128, 4 * 512], bf16, tag="masks", name="masks")
        itile = const.tile([128, 512], mybir.dt.int32, tag="itile", name="itile")
        nc.gpsimd.iota(itile[:], [[1, 512]], channel_multiplier=-1)
        for k in range(4):
            nc.vector.tensor_scalar(masks[:, ds(k * 512, 512)], itile[:],
                                    float(128 * k), None, ALU.is_ge)
        lnt = const.tile([128, (4 * L + 2) * CB], f32, tag="lnt", name="lnt")
        nc.sync.dma_start(lnt[:], lnp[:])

        hT = [ph.tile([128, T], f32, tag=f"h{cb}", name=f"h{cb}") for cb in range(CB)]

        def layernorm(idx_w, idx_b, dst):
            """h_T -> dst (6 x [128,1024] bf16). idx_* select lnt col groups."""
            hbf = []
            for cb in range(CB):
                t = phb.tile([128, T], bf16, tag=f"hb{cb}", name=f"hb{cb}")
                nc.vector.tensor_copy(t[:], hT[cb][:])
                hbf.append(t)
            for tg in range(NTG):
                sl = ds(tg * 512, 512)
                st0 = pst.tile([1, 512], f32, tag="st0", name="st0")
                st1 = pst.tile([1, 512], f32, tag="st1", name="st1")
                sq = []
                for cb in range(CB):
                    t = psq.tile([128, 512], bf16, tag="sq", name="sq")
                    nc.scalar.activation(t[:], hbf[cb][:, sl], AF.Square)
                    sq.append(t)
                for cb in range(CB):
                    nc.tensor.matmul(st0[:], ones[:], hbf[cb][:, sl],
                                     start=(cb == 0), stop=(cb == CB - 1))
                for cb in range(CB):
                    nc.tensor.matmul(st1[:], ones[:], sq[cb][:],
                                     start=(cb == 0), stop=(cb == CB - 1))
                mu = prow.tile([1, 512], f32, tag="mu", name="mu")
                nc.scalar.mul(mu[:], st0[:], 1.0 / C)
                musq = prow.tile([1, 512], f32, tag="musq", name="musq")
                nc.scalar.activation(musq[:], mu[:], AF.Square)
                var = prow.tile([1, 512], f32, tag="var", name="var")
                nc.vector.tensor_scalar(var[:], st1[:], 1.0 / C, None, ALU.mult)
                nc.vector.tensor_sub(var[:], var[:], musq[:])
                std = prow.tile([1, 512], f32, tag="std", name="std")
                nc.scalar.activation(std[:], var[:], AF.Sqrt, bias=eps1[:])
                rstd = prow.tile([1, 512], f32, tag="rstd", name="rstd")
                nc.vector.reciprocal(rstd[:], std[:])
                brf = prow.tile([1, 512], f32, tag="musq", name="brf")
                nc.vector.tensor_mul(brf[:], mu[:], rstd[:])
                # row-broadcast via K=1 matmul (PE) -> PSUM -> bf16 SBUF;
                # keeps gpsimd free for the weight-gather collectives
                abps = pst.tile([128, 512], f32, tag="st0", name="abps")
                nc.tensor.matmul(abps[:], onesr[:], rstd[:],
                                 start=True, stop=True)
                abc = pbc.tile([128, 512], bf16, tag="abc", name="abc")
                nc.vector.tensor_copy(abc[:], abps[:])
                bbps = pst.tile([128, 512], f32, tag="st1", name="bbps")
                nc.tensor.matmul(bbps[:], onesr[:], brf[:],
                                 start=True, stop=True)
                bbc = pbc.tile([128, 512], bf16, tag="bbc", name="bbc")
                nc.vector.tensor_copy(bbc[:], bbps[:])
                for cb in range(CB):
                    t1 = psq.tile([128, 512], bf16, tag="t1", name="t1")
                    nc.vector.tensor_mul(t1[:], hbf[cb][:, sl], abc[:])
                    nc.vector.tensor_sub(t1[:], t1[:], bbc[:])
                    nc.vector.tensor_scalar(
                        dst[cb][:, sl], t1[:],
                        lnt[:, ds(idx_w * CB + cb, 1)],
                        lnt[:, ds(idx_b * CB + cb, 1)],
                        ALU.mult, ALU.add)

        for rep in range(reps):
          for cb in range(CB):
            hb0 = phb.tile([128, T], bf16, tag=f"hb{cb}", name=f"h0b{cb}")
            nc.sync.dma_start(hb0[:], h0[:, ds(cb * T, T)])
            nc.vector.tensor_copy(hT[cb][:], hb0[:])
          for l in range(L):
            hn = [phn.tile([128, T], bf16, tag=f"hn{cb}", name=f"hn{cb}") for cb in range(CB)]
            layernorm(4 * l + 0, 4 * l + 1, hn)

            # ---- QK (transposed out) ----
            qT = [pqk.tile([128, T], bf16, tag=f"q{i}", name=f"q{i}") for i in range(CB)]
            kT = [pqk.tile([128, T], bf16, tag=f"k{i}", name=f"k{i}") for i in range(CB)]
            for db in range(12):
                wt = pwst.tile([128, 768], bf16, tag="wst", name="wst")
                nc.sync.dma_start(wt[:], wsl(l, OQK + db * 768, 768))
                for tg in range(NTG):
                    ps = pmm.tile([128, 512], f32, tag="mm", name="mm")
                    for cb in range(CB):
                        nc.tensor.matmul(ps[:], wt[:, ds(cb * 128, 128)],
                                         hn[cb][:, ds(tg * 512, 512)],
                                         start=(cb == 0), stop=(cb == CB - 1))
                    if db < 6:
                        nc.scalar.activation(qT[db][:, ds(tg * 512, 512)], ps[:],
                                             AF.Copy, scale=float(1.0 / np.sqrt(D)))
                    else:
                        nc.scalar.activation(kT[db - 6][:, ds(tg * 512, 512)],
                                             ps[:], AF.Copy)
            # ---- V (natural out, ones col per head) ----
            vA = [pv.tile([128, H * (D + 1)], bf16, tag=f"v{tt}", name=f"v{tt}") for tt in range(TT)]
            wvt = pwv.tile([128, CB * 768], bf16, tag="wv", name="wv")
            nc.sync.dma_start(wvt[:], wsl(l, OV, XV))
            for tt in range(TT):
                va3 = vA[tt].rearrange("p (h e) -> p h e", e=D + 1)
                nc.vector.memset(va3[:, :, D:D + 1], 1.0)
                for half in range(2):
                    w = 512 if half == 0 else 256
                    nh = w // D
                    ps = pmm.tile([128, 512], f32, tag="mm", name="mm")
                    for cb in range(CB):
                        nc.tensor.matmul(ps[:, 0:w],
                                         hn[cb][:, ds(tt * 128, 128)],
                                         wvt[:, ds(cb * 768 + half * 512, w)],
                                         start=(cb == 0), stop=(cb == CB - 1))
                    nc.vector.tensor_copy(
                        va3[:, ds(half * 8, nh), 0:D],
                        ps[:, 0:w].rearrange("p (h e) -> p h e", e=D))
            # ---- attention ----
            yT = [py.tile([128, T], bf16, tag=f"y{i}", name=f"y{i}") for i in range(CB)]
            items = [(hd, tg) for hd in range(H) for tg in range(NTG)]

            def att_stage_a(hd, tg):
                po = (hd % 2) * 64
                qs = qT[hd // 2][po:po + 64, :]
                ks = kT[hd // 2][po:po + 64, :]
                nsb = 4 * (tg + 1)
                ea = []
                for sb in range(nsb):
                    ps = pmm.tile([128, 512], f32, tag="mm", name="mm")
                    nc.tensor.matmul(ps[:], ks[:, ds(sb * 128, 128)],
                                     qs[:, ds(tg * 512, 512)],
                                     start=True, stop=True)
                    e = pexp.tile([128, 512], bf16, tag=f"e{sb}", name=f"e{sb}")
                    nc.scalar.activation(e[:], ps[:], AF.Exp)
                    kk = sb - 4 * tg
                    if kk >= 0:
                        nc.vector.tensor_mul(e[:], e[:],
                                             masks[:, ds(kk * 512, 512)])
                    ea.append(e)
                return ea

            def att_stage_b(hd, tg, ea):
                po = (hd % 2) * 64
                nsb = 4 * (tg + 1)
                yps = pmm.tile([128, 512], f32, tag="mm", name="mm")
                for sb in range(nsb):
                    nc.tensor.matmul(yps[0:65, :],
                                     vA[sb][:, ds(hd * 65, 65)], ea[sb][:],
                                     start=(sb == 0), stop=(sb == nsb - 1))
                iz = piz.tile([1, 512], f32, tag="iz", name="iz")
                nc.vector.reciprocal(iz[:], yps[64:65, :])
                izps = pst.tile([64, 512], f32, tag="st1", name="izps")
                nc.tensor.matmul(izps[:], onesr[:, 0:64], iz[:],
                                 start=True, stop=True)
                izb = pbc.tile([64, 512], bf16, tag="izb", name="izb")
                nc.vector.tensor_copy(izb[:], izps[:])
                nc.vector.tensor_mul(
                    yT[hd // 2][po:po + 64, ds(tg * 512, 512)],
                    yps[0:64, :], izb[:])

            prev = None
            for it in items:
                ea = att_stage_a(*it)
                if prev is not None:
                    att_stage_b(prev[0][0], prev[0][1], prev[1])
                prev = (it, ea)
            att_stage_b(prev[0][0], prev[0][1], prev[1])
            # ---- attn proj + residual ----
            for cb in range(CB):
                wt = pwst.tile([128, 768], bf16, tag="wst", name="wst")
                nc.sync.dma_start(wt[:], wsl(l, OPJ + cb * 768, 768))
                for tg in range(NTG):
                    ps = pmm.tile([128, 512], f32, tag="mm", name="mm")
                    for k in range(CB):
                        nc.tensor.matmul(ps[:], wt[:, ds(k * 128, 128)],
                                         yT[k][:, ds(tg * 512, 512)],
                                         start=(k == 0), stop=(k == CB - 1))
                    nc.vector.tensor_add(hT[cb][:, ds(tg * 512, 512)],
                                         hT[cb][:, ds(tg * 512, 512)], ps[:])
            # ---- LN2 + MLP ----
            layernorm(4 * l + 2, 4 * l + 3, hn)
            for tg in range(NTG):
                sl = ds(tg * 512, 512)
                gl = []
                for fb in range(FBL):
                    wt = pwst.tile([128, 768], bf16, tag="wst", name="wst")
                    nc.sync.dma_start(wt[:], wsl(l, OFC + fb * 768, 768))
                    ps = pmm.tile([128, 512], f32, tag="mm", name="mm")
                    for cb in range(CB):
                        nc.tensor.matmul(ps[:], wt[:, ds(cb * 128, 128)],
                                         hn[cb][:, sl],
                                         start=(cb == 0), stop=(cb == CB - 1))
                    g = pg.tile([128, 512], bf16, tag=f"g{fb}", name=f"g{fb}")
                    nc.scalar.activation(g[:], ps[:], AF.Gelu_apprx_tanh)
                    gl.append(g)
                for cb in range(CB):
                    wt = pwm.tile([128, F], bf16, tag="wm", name="wm")
                    nc.sync.dma_start(wt[:], wsl(l, OMP + cb * F, F))
                    ps = pmm.tile([128, 512], f32, tag="mm", name="mm")
                    for fb in range(FBL):
                        nc.tensor.matmul(ps[:], wt[:, ds(fb * 128, 128)],
                                         gl[fb][:],
                                         start=(fb == 0), stop=(fb == FBL - 1))
                    resid_last = nc.vector.tensor_add(
                        hT[cb][:, sl], hT[cb][:, sl], ps[:])
            if use_ag and rep == 0:
                # queue the next layer-pair's gather after this pair's first
                # layer; the lm gather goes out at layer 8 for extra slack
                if l % 2 == 0 and l // 2 + 1 < L // 2:
                    emit_ag(l // 2 + 1)
                if l == 8:
                    glm = emit_ag_lm()

        # ---- final LN + lm head (uint8-quantized logits) ----
        hf = [phn.tile([128, T], bf16, tag=f"hn{cb}", name=f"hn{cb}") for cb in range(CB)]
        layernorm(4 * L, 4 * L + 1, hf)
        # scale accumulators reuse the vA slots (attention is over by now)
        sclb = [pv.tile([128, NVC], f32, tag=f"v{tt}", name=f"scl{tt}")
                for tt in range(TT)]
        for vc in range(NVC):
            lt = plm.tile([128, CB * 512], bf16, tag="lm", name="lm")
            ltd = nc.sync.dma_start(lt[:], glm[:, ds(vc * CB * 512, CB * 512)])
            if vc < 4:
                # pin the first lm weight loads behind the last layer's
                # residual: stops the scheduler hoisting them to the front of
                # a DMA queue, where their late-arriving AllGather dependency
                # head-of-line-blocks every weight load queued behind them
                add_dep_helper(ltd.ins, resid_last.ins, sync=False,
                               reason="defer lm loads past layer stack")
            for tt in range(TT):
                ps = pmm.tile([128, 512], f32, tag="mm", name="mm")
                for cb in range(CB):
                    nc.tensor.matmul(ps[:], hf[cb][:, ds(tt * 128, 128)],
                                     lt[:, ds(cb * 512, 512)],
                                     start=(cb == 0), stop=(cb == CB - 1))
                amax = prow.tile([128, 1], f32, tag="amax", name="amax")
                nc.vector.tensor_reduce(amax[:], ps[:], AX.X, ALU.max,
                                        apply_absolute_value=True)
                # scale = amax/127 (eps-clamped so reciprocal stays finite)
                nc.vector.tensor_scalar(sclb[tt][:, ds(vc, 1)], amax[:],
                                        1.0 / 127.0, 1e-32, ALU.mult, ALU.max)
                sinv = prow.tile([128, 1], f32, tag="sinv", name="sinv")
                nc.vector.reciprocal(sinv[:], sclb[tt][:, ds(vc, 1)])
                # offset to unsigned range; hw convert rounds to nearest, so
                # +128.0 exactly (adding .5 would put every value on a tie)
                qt = psq.tile([128, 512], u8, tag="t1", name="qt")
                nc.vector.tensor_scalar(qt[:], ps[:], sinv[:], 128.0,
                                        ALU.mult, ALU.add)
                nc.sync.dma_start(out[ds(tt * 128, 128), ds(vc * 512, 512)],
                                  qt[:])
        for tt in range(TT):
            nc.sync.dma_start(osc[ds(tt * 128, 128), :], sclb[tt][:])

    nc.compile()
    return nc


def _pack_stationary(w, nblk):
    kb = w.shape[0] // 128
    t = w.reshape(kb, 128, nblk, 128)
    return np.ascontiguousarray(
        t.transpose(1, 2, 0, 3).reshape(128, nblk * kb * 128))


def _prep(inputs, use_ag=True):
    wte = np.asarray(inputs["wte"], np.float32)
    wpe = np.asarray(inputs["wpe"], np.float32)
    x = np.asarray(inputs["x"])
    aw = np.asarray(inputs["attn_w"], np.float32)
    pw = np.asarray(inputs["attnp_w"], np.float32)
    fw = np.asarray(inputs["fc_w"], np.float32)
    mw = np.asarray(inputs["mproj_w"], np.float32)
    lm = np.asarray(inputs["lm_w"], np.float32)
    for nm in ("attn_b", "attnp_b", "fc_b", "mproj_b"):
        assert not np.any(np.asarray(inputs[nm])), f"{nm} nonzero; unsupported"

    # bundled packed layer weights: [128, L * XL], per layer qk|v|proj|fc|mproj
    blocks = []
    for l in range(L):
        blocks.append(np.concatenate([
            _pack_stationary(aw[l][:, :2 * C], 12),
            np.ascontiguousarray(
                aw[l][:, 2 * C:].reshape(CB, 128, C).transpose(1, 0, 2)
                .reshape(128, CB * C)),
            _pack_stationary(pw[l], CB),
            _pack_stationary(fw[l], FBL),
            _pack_stationary(mw[l], CB),
        ], axis=1))
    wl_full = np.concatenate(blocks, axis=1).astype(BF16)

    lmp = np.zeros((C, VPAD), np.float32)
    lmp[:, :V] = lm
    wlm_halves = []
    for vh in range(2):
        t = lmp[:, vh * VSH:(vh + 1) * VSH].reshape(CB, 128, NVC, 512)
        wlm_halves.append(np.ascontiguousarray(
            t.transpose(1, 2, 0, 3).reshape(128, NVC * CB * 512)).astype(BF16))

    lncols = np.zeros((128, (4 * L + 2) * CB), np.float32)
    names = [("ln1_w", 0), ("ln1_b", 1), ("ln2_w", 2), ("ln2_b", 3)]
    for l in range(L):
        for nm, k in names:
            vec = np.asarray(inputs[nm], np.float32)[l]
            lncols[:, (4 * l + k) * CB:(4 * l + k + 1) * CB] = \
                vec.reshape(CB, 128).T
    lncols[:, 4 * L * CB:(4 * L + 1) * CB] = \
        np.asarray(inputs["lnf_w"], np.float32).reshape(CB, 128).T
    lncols[:, (4 * L + 1) * CB:] = \
        np.asarray(inputs["lnf_b"], np.float32).reshape(CB, 128).T

    h0s = []
    for b in range(B):
        h = wte[x[b]] + wpe[:T]
        hTr = np.ascontiguousarray(
            h.T.reshape(CB, 128, T).transpose(1, 0, 2).reshape(128, CB * T))
        h0s.append(hTr.astype(BF16))

    in_maps = []
    for c in range(8):
        b = c % 4          # batch elem
        vh = c // 4        # vocab half
        p4 = c % 4         # position within the 4-rank lm gather group
        if use_ag:
            # [L*16, XL]: rows 16l:16l+16 = this core's shard of layer l
            shard = np.ascontiguousarray(
                wl_full[16 * c:16 * c + 16].reshape(16, L, XL)
                .transpose(1, 0, 2).reshape(L * 16, XL))
            in_maps.append({
                "h0": h0s[b],
                "wl": shard,
                "wlm": np.ascontiguousarray(wlm_halves[vh][32 * p4:32 * p4 + 32]),
                "lnp": lncols,
            })
        else:
            in_maps.append({
                "h0": h0s[b], "wl": wl_full, "wlm": wlm_halves[vh],
                "lnp": lncols,
            })
    return in_maps


def kernel(**inputs):
    from concourse import bass_utils
    if "nc" not in _CACHE:
        t0 = time.time()
        _CACHE["nc"] = _build()
        print(f"[kernel] build+compile {time.time()-t0:.1f}s", file=sys.stderr)
    nc = _CACHE["nc"]
    in_maps = _prep(inputs)
    res = bass_utils.run_bass_kernel_spmd(nc, in_maps, core_ids=list(range(8)))

    def dequant(r):
        q = r["out"].reshape(T, NVC, 512).astype(np.float32)
        s = r["osc"].reshape(T, NVC, 1)
        return ((q - 128.0) * s).reshape(T, VSH)

    outs = [dequant(r) for r in res.results]
    full = np.empty((B, T, V), np.float32)
    for b in range(B):
        full[b] = np.concatenate([outs[b], outs[b + 4]], axis=1)[:, :V]
    return full
